# revision 12
# baseline (speedup 1.0000x reference)
"""GQA attention block (RoPE + causal attention + output proj) on 8 TRN2 NeuronCores.

Sharding: batch (B=2) x kv-head-group (KVH=4) -> 8 cores.
Core c handles batch b=c//4, kv group g=c%4 (q heads 4g..4g+3, kv head g).
Per-core tensor-parallel attention; AllGather of per-head outputs within each
batch's 4-core group; column-split wo after the gather.

All matmuls run in bf16 (fp32 PSUM accumulation). Layouts are transposed
([feature, token]) so Q/K/V projections, scores (computed as S^T = K-stationary),
and PV all feed the TensorEngine without transposes; softmax runs without
max-subtraction (logits are provably small for this problem's scale).

Pipelining: PV runs one k-tile behind scores/exp; each span's output projection
is deferred until after the next span's attention so the AllGather latency hides
behind compute.
"""

import sys

if "/opt/trn_rl_repo" not in sys.path:
    sys.path.insert(0, "/opt/trn_rl_repo")

import numpy as np
import ml_dtypes

import concourse.bass as bass
import concourse.mybir as mybir
import concourse.tile as tile
from concourse import bacc
from concourse.bass_utils import run_bass_kernel_spmd

BF16 = ml_dtypes.bfloat16

B, S, HID = 2, 2048, 1024
H, KVH, D = 16, 4, 64
G = H // KVH
N_CORES = 8
SPAN = 512
NSPAN = S // SPAN  # 4
NCH = HID // 128  # 8 contraction chunks
NKT = S // 128  # 16 k-tiles
F32 = mybir.dt.float32
BF = mybir.dt.bfloat16

TRACE = False
_CACHED = {}


def _build_nc():
    nc = bacc.Bacc("TRN2", target_bir_lowering=False, debug=False, num_devices=N_CORES)

    xT = nc.dram_tensor("xT", [HID, S], BF, kind="ExternalInput")
    wq = nc.dram_tensor("wq", [HID, 256], BF, kind="ExternalInput")
    wkv = nc.dram_tensor("wkv", [HID, 128], BF, kind="ExternalInput")
    wo = nc.dram_tensor("wo", [HID, 256], BF, kind="ExternalInput")
    c2 = nc.dram_tensor("c2", [128, S], BF, kind="ExternalInput")
    s2 = nc.dram_tensor("s2", [128, S], BF, kind="ExternalInput")
    c1 = nc.dram_tensor("c1", [64, S], BF, kind="ExternalInput")
    s1 = nc.dram_tensor("s1", [64, S], BF, kind="ExternalInput")
    ident = nc.dram_tensor("ident", [128, 128], BF, kind="ExternalInput")
    out = nc.dram_tensor("out", [256, S], F32, kind="ExternalOutput")

    EXP = mybir.ActivationFunctionType.Exp
    LN = mybir.ActivationFunctionType.Ln

    with tile.TileContext(nc) as tc:
        with (
            tc.tile_pool(name="main", bufs=1) as main,
            tc.tile_pool(name="dramp", bufs=1, space="DRAM") as dramp,
        ):
            # ---- persistent SBUF; per-chunk input tiles so compute can start
            # as soon as each chunk's DMA lands ----
            xT_sb = [main.tile([128, S], BF, name=f"xT{k}") for k in range(NCH)]
            wq_sb = [main.tile([128, 256], BF, name=f"wq{k}") for k in range(NCH)]
            wkv_sb = [main.tile([128, 128], BF, name=f"wkv{k}") for k in range(NCH)]
            wo_sb = [main.tile([128, 256], BF, name=f"wo{k}") for k in range(NCH)]
            c2_sb = main.tile([128, S], BF, name="c2_sb")
            s2_sb = main.tile([128, S], BF, name="s2_sb")
            c1_sb = main.tile([64, S], BF, name="c1_sb")
            s1_sb = main.tile([64, S], BF, name="s1_sb")
            ident_sb = main.tile([128, 128], BF, name="ident_sb")
            qT0_sb = main.tile([128, S], BF, name="qT0_sb")
            qT1_sb = main.tile([128, S], BF, name="qT1_sb")
            kT2_sb = main.tile([128, S], BF, name="kT2_sb")
            vT_sb = main.tile([64, S], BF, name="vT_sb")
            vaug_sb = main.tile([128, NKT, 65], BF, name="vaug_sb")
            ones_sb = main.tile([1, 64], BF, name="ones_sb")
            qT_sb = [qT0_sb, qT1_sb]

            # tiny warmup AllGather: absorbs ncfw init + inter-core alignment
            wuin = dramp.tile([128, 16], BF, name="wuin")
            wuout = dramp.tile([512, 16], BF, name="wuout")
            nc.gpsimd.collective_compute(
                "AllGather",
                mybir.AluOpType.bypass,
                replica_groups=[[0, 1, 2, 3], [4, 5, 6, 7]],
                ins=[wuin[:].opt()],
                outs=[wuout[:].opt()],
            )
            for k in range(NCH):
                nc.sync.dma_start(xT_sb[k][:], xT[128 * k : 128 * k + 128, :])
                nc.sync.dma_start(wkv_sb[k][:], wkv[128 * k : 128 * k + 128, :])
            for k in range(NCH):
                nc.sync.dma_start(wq_sb[k][:], wq[128 * k : 128 * k + 128, :])
            nc.sync.dma_start(c1_sb[:], c1[:])
            nc.sync.dma_start(s1_sb[:], s1[:])
            nc.sync.dma_start(c2_sb[:], c2[:])
            nc.sync.dma_start(s2_sb[:], s2[:])
            nc.sync.dma_start(ident_sb[:], ident[:])
            for k in range(NCH):
                nc.sync.dma_start(wo_sb[k][:], wo[128 * k : 128 * k + 128, :])
            nc.vector.memset(ones_sb[:], 1.0)

            # ---- phase 1: projections (transposed layout) + RoPE; KV first so
            # the V-transpose can run while the Q projections are still going ----
            HS = S // 2  # phase-1 half-sequence granularity (2 PSUM banks)
            with (
                tc.tile_pool(name="psA", bufs=2, space="PSUM") as psA,
                tc.tile_pool(name="ropep", bufs=2) as ropep,
                tc.tile_pool(name="psT", bufs=2, space="PSUM") as psT,
            ):
                for hf in range(2):
                    f0 = HS * hf
                    kvp = psA.tile([128, HS], F32, tag="qkv", name=f"kvp{hf}")
                    for sp in range(2):
                        for k in range(NCH):
                            nc.tensor.matmul(
                                kvp[:, SPAN * sp : SPAN * (sp + 1)],
                                wkv_sb[k][:],
                                xT_sb[k][:, f0 + SPAN * sp : f0 + SPAN * (sp + 1)],
                                start=(k == 0),
                                stop=(k == NCH - 1),
                            )
                    kb = ropep.tile([64, HS], BF, tag="kb", name=f"kb{hf}")
                    nc.scalar.copy(kb[:], kvp[0:64, :])
                    nc.scalar.copy(vT_sb[:, f0 : f0 + HS], kvp[64:128, :])
                    tcosk = ropep.tile([64, HS], BF, tag="tcos", name=f"tcosk{hf}")
                    tsink = ropep.tile([64, HS], BF, tag="tsin", name=f"tsink{hf}")
                    nc.vector.tensor_mul(tcosk[:], kb[:], c1_sb[:, f0 : f0 + HS])
                    for dst, src in ((0, 32), (32, 0)):
                        nc.vector.tensor_mul(
                            tsink[dst : dst + 32, :],
                            kb[src : src + 32, :],
                            s1_sb[src : src + 32, f0 : f0 + HS],
                        )
                    nc.vector.tensor_add(
                        kT2_sb[0:64, f0 : f0 + HS], tcosk[:], tsink[:]
                    )
                    nc.vector.tensor_copy(
                        kT2_sb[64:128, f0 : f0 + HS], kT2_sb[0:64, f0 : f0 + HS]
                    )
                    # V transpose to [token, d] for this half
                    for t in range(8 * hf, 8 * hf + 8):
                        trp = psT.tile([128, 64], BF, tag="tr", name=f"tr{t}")
                        nc.tensor.transpose(
                            trp[:],
                            vT_sb[:, 128 * t : 128 * (t + 1)],
                            ident_sb[0:64, 0:64],
                        )
                        nc.vector.tensor_copy(vaug_sb[:, t, 0:64], trp[:])
                nc.vector.memset(vaug_sb[:, :, 64:65], 1.0)

                for hf in range(2):
                    for p in range(2):
                        f0 = HS * hf
                        qp = psA.tile([128, HS], F32, tag="qkv", name=f"qp{p}_{hf}")
                        for sp in range(2):
                            for k in range(NCH):
                                nc.tensor.matmul(
                                    qp[:, SPAN * sp : SPAN * (sp + 1)],
                                    wq_sb[k][:, 128 * p : 128 * (p + 1)],
                                    xT_sb[k][:, f0 + SPAN * sp : f0 + SPAN * (sp + 1)],
                                    start=(k == 0),
                                    stop=(k == NCH - 1),
                                )
                        qb = ropep.tile([128, HS], BF, tag="qb", name=f"qb{p}{hf}")
                        nc.scalar.copy(qb[:], qp[:])
                        tcos = ropep.tile([128, HS], BF, tag="tcos", name=f"tc{p}{hf}")
                        tsin = ropep.tile([128, HS], BF, tag="tsin", name=f"ts{p}{hf}")
                        nc.vector.tensor_mul(tcos[:], qb[:], c2_sb[:, f0 : f0 + HS])
                        for dst, src in ((0, 32), (32, 0), (64, 96), (96, 64)):
                            nc.vector.tensor_mul(
                                tsin[dst : dst + 32, :],
                                qb[src : src + 32, :],
                                s2_sb[src : src + 32, f0 : f0 + HS],
                            )
                        nc.vector.tensor_add(
                            qT_sb[p][:, f0 : f0 + HS], tcos[:], tsin[:]
                        )

            # ---- phase 3: attention spans, AllGather, output projection ----
            with (
                tc.tile_pool(name="psS", bufs=2, space="PSUM") as psS,
                tc.tile_pool(name="psO", bufs=1, space="PSUM") as psO,
                tc.tile_pool(name="pp", bufs=5) as pp,
                tc.tile_pool(name="work", bufs=2) as work,
            ):
                rg = [[0, 1, 2, 3], [4, 5, 6, 7]]
                pending_oproj = []

                for J in range(NSPAN):
                    q0 = SPAN * J
                    nkt_j = 4 * (J + 1)
                    oproj_ready = pending_oproj
                    pending_oproj = []
                    opsum = psO.tile([128, 4 * SPAN], F32, tag="o", name=f"opsum{J}")

                    prev_pv = None  # (j, [pt_pr0, pt_pr1], col offset)

                    def emit_pv(j, pts, off):
                        for pr in range(2):
                            for hh in range(2):
                                h = 2 * pr + hh
                                nc.tensor.matmul(
                                    opsum[0:65, SPAN * h + off : SPAN * (h + 1)],
                                    vaug_sb[:, j, :],
                                    pts[pr][:, SPAN * hh + off : SPAN * (hh + 1)],
                                    start=(j == 0),
                                    stop=(j == nkt_j - 1),
                                )

                    for j in range(nkt_j):
                        jj = j - 4 * J  # >= 0 on causal-boundary k-tiles
                        off = 128 * jj if jj > 0 else 0
                        pts = []
                        for pr in range(2):
                            sps = psS.tile(
                                [128, 2 * SPAN], F32, tag="s", name=f"s{J}_{j}_{pr}"
                            )
                            pt = pp.tile(
                                [128, 2 * SPAN], BF, tag="p", name=f"p{J}_{j}_{pr}"
                            )
                            pts.append(pt)
                            src = qT_sb[pr]
                            nc.tensor.matmul(
                                sps[:, off:SPAN],
                                kT2_sb[0:64, 128 * j : 128 * (j + 1)],
                                src[0:64, q0 + off : q0 + SPAN],
                                start=True,
                                stop=True,
                            )
                            nc.tensor.matmul(
                                sps[:, SPAN + off : 2 * SPAN],
                                kT2_sb[64:128, 128 * j : 128 * (j + 1)],
                                src[64:128, q0 + off : q0 + SPAN],
                                start=True,
                                stop=True,
                            )
                            # exp over the two valid column blocks (strided AP)
                            nc.scalar.activation(
                                pt[:].rearrange("p (h q) -> p h q", h=2)[
                                    :, :, off:SPAN
                                ],
                                sps[:].rearrange("p (h q) -> p h q", h=2)[
                                    :, :, off:SPAN
                                ],
                                EXP,
                            )
                            if jj >= 0:
                                # causal triangle on the diagonal 128-block
                                nc.gpsimd.affine_select(
                                    pt[:].rearrange("p (h q) -> p h q", h=2)[
                                        :, :, off : off + 128
                                    ],
                                    pt[:].rearrange("p (h q) -> p h q", h=2)[
                                        :, :, off : off + 128
                                    ],
                                    pattern=[[0, 2], [1, 128]],
                                    compare_op=mybir.AluOpType.is_ge,
                                    fill=0.0,
                                    base=0,
                                    channel_multiplier=-1,
                                )
                        if prev_pv is not None:
                            emit_pv(*prev_pv)
                        prev_pv = (j, pts, off)
                        if j == 5:
                            for fn in oproj_ready:
                                fn()
                            oproj_ready = []
                    emit_pv(*prev_pv)

                    # normalization + AllGather + output projection, emitted
                    # deferred (inside the next span's j-loop) so the ACT
                    # recip chain and the AllGather hide behind attention
                    def make_norm_ag(J=J, q0=q0, opsum=opsum):
                        def _norm():
                            # denominator row -> SBUF bf16 (for the broadcast MM)
                            dsb = work.tile([1, 4 * SPAN], BF, tag="dsb", name=f"dsb{J}")
                            nc.scalar.copy(dsb[:], opsum[64:65, :])

                            def _bcag():
                                agin = dramp.tile([256, SPAN], BF, name=f"agin{J}")
                                agout = dramp.tile([4 * 256, SPAN], BF, name=f"agout{J}")
                                for h in range(4):
                                    bc = psS.tile(
                                        [64, SPAN], F32, tag="s", name=f"bc{J}_{h}"
                                    )
                                    nc.tensor.matmul(
                                        bc[:],
                                        ones_sb[:],
                                        dsb[0:1, SPAN * h : SPAN * (h + 1)],
                                        start=True,
                                        stop=True,
                                    )
                                    rec = work.tile(
                                        [64, SPAN], F32, tag="rec", name=f"rec{J}_{h}"
                                    )
                                    nc.vector.reciprocal_approx_fast(rec[:], bc[:])
                                    onrm = work.tile(
                                        [64, SPAN], BF, tag="onrm", name=f"on{J}_{h}"
                                    )
                                    nc.vector.tensor_mul(
                                        onrm[:],
                                        opsum[0:64, SPAN * h : SPAN * (h + 1)],
                                        rec[:],
                                    )
                                    nc.sync.dma_start(
                                        agin[64 * h : 64 * (h + 1), :], onrm[:]
                                    )

                                nc.gpsimd.collective_compute(
                                    "AllGather",
                                    mybir.AluOpType.bypass,
                                    replica_groups=rg,
                                    ins=[agin[:].opt()],
                                    outs=[agout[:].opt()],
                                )
                                ofull = work.tile(
                                    [128, NCH, SPAN], BF, tag="ofull", bufs=3, name=f"of{J}"
                                )
                                for k in range(NCH):
                                    nc.sync.dma_start(
                                        ofull[:, k, :], agout[128 * k : 128 * (k + 1), :]
                                    )

                                def _oproj():
                                    for half in range(2):
                                        po = psS.tile(
                                            [128, SPAN], F32, tag="s", name=f"po{J}_{half}"
                                        )
                                        for k in range(NCH):
                                            nc.tensor.matmul(
                                                po[:],
                                                wo_sb[k][
                                                    :, 128 * half : 128 * (half + 1)
                                                ],
                                                ofull[:, k, :],
                                                start=(k == 0),
                                                stop=(k == NCH - 1),
                                            )
                                        outT = work.tile(
                                            [128, SPAN],
                                            F32,
                                            tag="outT",
                                            name=f"ot{J}_{half}",
                                        )
                                        nc.vector.tensor_copy(outT[:], po[:])
                                        nc.sync.dma_start(
                                            out[
                                                128 * half : 128 * (half + 1),
                                                q0 : q0 + SPAN,
                                            ],
                                            outT[:],
                                        )

                                pending_oproj.append(_oproj)

                            _bcag()

                        return _norm

                    make_norm_ag()()

                for fn in oproj_ready:
                    fn()
                for fn in pending_oproj:
                    fn()

    nc.finalize()
    return nc


def _host_inputs(x, cos, sin, wq, wk, wv, wo):
    cosT = np.ascontiguousarray(cos.T).astype(np.float32)  # [64, S]
    sinT = np.ascontiguousarray(sin.T).astype(np.float32)
    s1n = np.concatenate([-sinT[0:32], sinT[32:64]], axis=0)  # [64, S]
    c2n = np.concatenate([cosT, cosT], axis=0).astype(BF16)  # [128, S]
    # partition-swapped: row p holds the sin factor for the partner row p^32,
    # so both DVE operands read from the same base partition
    s1w = np.concatenate([s1n[32:64], s1n[0:32]], axis=0)
    s2w = np.concatenate([s1w, s1w], axis=0).astype(BF16)
    cosT = cosT.astype(BF16)
    s1w = s1w.astype(BF16)
    ident = np.eye(128, dtype=BF16)

    in_maps = []
    for c in range(N_CORES):
        b, g = c // 4, c % 4
        xT = np.ascontiguousarray(x[b].T).astype(BF16)
        wq_c = np.ascontiguousarray(wq[:, 256 * g : 256 * (g + 1)] / 8.0).astype(BF16)
        wkv_c = np.ascontiguousarray(
            np.concatenate(
                [wk[:, 64 * g : 64 * (g + 1)], wv[:, 64 * g : 64 * (g + 1)]], axis=1
            )
        ).astype(BF16)
        wo_c = np.ascontiguousarray(wo[:, 256 * g : 256 * (g + 1)]).astype(BF16)
        in_maps.append(
            {
                "xT": xT,
                "wq": wq_c,
                "wkv": wkv_c,
                "wo": wo_c,
                "c2": c2n,
                "s2": s2w,
                "c1": cosT,
                "s1": s1w,
                "ident": ident,
            }
        )
    return in_maps


def kernel(x, cos, sin, wq, wk, wv, wo):
    if "nc" not in _CACHED:
        _CACHED["nc"] = _build_nc()
    nc = _CACHED["nc"]
    in_maps = _host_inputs(
        np.asarray(x, np.float32),
        np.asarray(cos, np.float32),
        np.asarray(sin, np.float32),
        np.asarray(wq, np.float32),
        np.asarray(wk, np.float32),
        np.asarray(wv, np.float32),
        np.asarray(wo, np.float32),
    )
    res = run_bass_kernel_spmd(
        nc, in_maps, core_ids=list(range(N_CORES)), trace=TRACE
    )
    _CACHED["last_result"] = res
    out = np.empty((B, S, HID), dtype=np.float32)
    for c in range(N_CORES):
        b, g = c // 4, c % 4
        out[b, :, 256 * g : 256 * (g + 1)] = res.results[c]["out"].T
    return out


# revision 13
# speedup vs baseline: 1.1640x; 1.1640x over previous
"""GQA attention block (RoPE + causal attention + output proj) on 8 TRN2 NeuronCores.

Sharding: batch (B=2) x kv-head-group (KVH=4) -> 8 cores.
Core c handles batch b=c//4, kv group g=c%4 (q heads 4g..4g+3, kv head g).
Per-core tensor-parallel attention; AllGather of per-head outputs within each
batch's 4-core group; column-split wo after the gather.

All matmuls run in bf16 (fp32 PSUM accumulation). Layouts are transposed
([feature, token]) so Q/K/V projections, scores (computed as S^T = K-stationary),
and PV all feed the TensorEngine without transposes; softmax runs without
max-subtraction (logits are provably small for this problem's scale).

Pipelining: PV runs one k-tile behind scores/exp; each span's output projection
is deferred until after the next span's attention so the AllGather latency hides
behind compute.
"""

import sys

if "/opt/trn_rl_repo" not in sys.path:
    sys.path.insert(0, "/opt/trn_rl_repo")

import numpy as np
import ml_dtypes

import concourse.bass as bass
import concourse.mybir as mybir
import concourse.tile as tile
from concourse import bacc
from concourse.bass_utils import run_bass_kernel_spmd

BF16 = ml_dtypes.bfloat16

B, S, HID = 2, 2048, 1024
H, KVH, D = 16, 4, 64
G = H // KVH
N_CORES = 8
SPAN = 512
NSPAN = S // SPAN  # 4
NCH = HID // 128  # 8 contraction chunks
NKT = S // 128  # 16 k-tiles
F32 = mybir.dt.float32
BF = mybir.dt.bfloat16

TRACE = False
_CACHED = {}


def _build_nc():
    nc = bacc.Bacc("TRN2", target_bir_lowering=False, debug=False, num_devices=N_CORES)

    xT = nc.dram_tensor("xT", [HID, S], BF, kind="ExternalInput")
    wq = nc.dram_tensor("wq", [HID, 256], BF, kind="ExternalInput")
    wkv = nc.dram_tensor("wkv", [HID, 128], BF, kind="ExternalInput")
    wo = nc.dram_tensor("wo", [HID, 256], BF, kind="ExternalInput")
    c2 = nc.dram_tensor("c2", [128, S], BF, kind="ExternalInput")
    s2 = nc.dram_tensor("s2", [128, S], BF, kind="ExternalInput")
    c1 = nc.dram_tensor("c1", [64, S], BF, kind="ExternalInput")
    s1 = nc.dram_tensor("s1", [64, S], BF, kind="ExternalInput")
    ident = nc.dram_tensor("ident", [128, 128], BF, kind="ExternalInput")
    out = nc.dram_tensor("out", [256, S], F32, kind="ExternalOutput")

    EXP = mybir.ActivationFunctionType.Exp
    LN = mybir.ActivationFunctionType.Ln

    with tile.TileContext(nc) as tc:
        with (
            tc.tile_pool(name="main", bufs=1) as main,
            tc.tile_pool(name="dramp", bufs=1, space="DRAM") as dramp,
        ):
            # ---- persistent SBUF; per-chunk input tiles so compute can start
            # as soon as each chunk's DMA lands ----
            xT_sb = [main.tile([128, S], BF, name=f"xT{k}") for k in range(NCH)]
            wq_sb = [main.tile([128, 256], BF, name=f"wq{k}") for k in range(NCH)]
            wkv_sb = [main.tile([128, 128], BF, name=f"wkv{k}") for k in range(NCH)]
            wo_sb = [main.tile([128, 256], BF, name=f"wo{k}") for k in range(NCH)]
            c2_sb = main.tile([128, S], BF, name="c2_sb")
            s2_sb = main.tile([128, S], BF, name="s2_sb")
            c1_sb = main.tile([64, S], BF, name="c1_sb")
            s1_sb = main.tile([64, S], BF, name="s1_sb")
            ident_sb = main.tile([128, 128], BF, name="ident_sb")
            qT0_sb = main.tile([128, S], BF, name="qT0_sb")
            qT1_sb = main.tile([128, S], BF, name="qT1_sb")
            kT2_sb = main.tile([128, S], BF, name="kT2_sb")
            vT_sb = main.tile([64, S], BF, name="vT_sb")
            vaug_sb = main.tile([128, NKT, 65], BF, name="vaug_sb")
            ones_sb = main.tile([1, 64], BF, name="ones_sb")
            qT_sb = [qT0_sb, qT1_sb]

            # tiny warmup AllGather: absorbs ncfw init + inter-core alignment
            wuin = dramp.tile([128, 16], BF, name="wuin")
            wuout = dramp.tile([512, 16], BF, name="wuout")
            wu_cc = nc.gpsimd.collective_compute(
                "AllGather",
                mybir.AluOpType.bypass,
                replica_groups=[[0, 1, 2, 3], [4, 5, 6, 7]],
                ins=[wuin[:].opt()],
                outs=[wuout[:].opt()],
            )
            first_gpsimd = [wu_cc]
            for k in range(NCH):
                nc.sync.dma_start(xT_sb[k][:], xT[128 * k : 128 * k + 128, :])
                nc.sync.dma_start(wkv_sb[k][:], wkv[128 * k : 128 * k + 128, :])
            for k in range(NCH):
                nc.sync.dma_start(wq_sb[k][:], wq[128 * k : 128 * k + 128, :])
            nc.sync.dma_start(c1_sb[:], c1[:])
            nc.sync.dma_start(s1_sb[:], s1[:])
            nc.sync.dma_start(c2_sb[:], c2[:])
            nc.sync.dma_start(s2_sb[:], s2[:])
            nc.sync.dma_start(ident_sb[:], ident[:])
            for k in range(NCH):
                nc.sync.dma_start(wo_sb[k][:], wo[128 * k : 128 * k + 128, :])
            nc.vector.memset(ones_sb[:], 1.0)

            # ---- phase 1: projections (transposed layout) + RoPE; KV first so
            # the V-transpose can run while the Q projections are still going ----
            HS = S // 2  # phase-1 half-sequence granularity (2 PSUM banks)
            with (
                tc.tile_pool(name="psA", bufs=2, space="PSUM") as psA,
                tc.tile_pool(name="ropep", bufs=2) as ropep,
                tc.tile_pool(name="psT", bufs=2, space="PSUM") as psT,
            ):
                for hf in range(2):
                    f0 = HS * hf
                    kvp = psA.tile([128, HS], F32, tag="qkv", name=f"kvp{hf}")
                    for sp in range(2):
                        for k in range(NCH):
                            nc.tensor.matmul(
                                kvp[:, SPAN * sp : SPAN * (sp + 1)],
                                wkv_sb[k][:],
                                xT_sb[k][:, f0 + SPAN * sp : f0 + SPAN * (sp + 1)],
                                start=(k == 0),
                                stop=(k == NCH - 1),
                            )
                    kb = ropep.tile([64, HS], BF, tag="kb", name=f"kb{hf}")
                    nc.scalar.copy(kb[:], kvp[0:64, :])
                    nc.scalar.copy(vT_sb[:, f0 : f0 + HS], kvp[64:128, :])
                    tcosk = ropep.tile([64, HS], BF, tag="tcos", name=f"tcosk{hf}")
                    tsink = ropep.tile([64, HS], BF, tag="tsin", name=f"tsink{hf}")
                    nc.vector.tensor_mul(tcosk[:], kb[:], c1_sb[:, f0 : f0 + HS])
                    for dst, src in ((0, 32), (32, 0)):
                        nc.vector.tensor_mul(
                            tsink[dst : dst + 32, :],
                            kb[src : src + 32, :],
                            s1_sb[src : src + 32, f0 : f0 + HS],
                        )
                    nc.vector.tensor_add(
                        kT2_sb[0:64, f0 : f0 + HS], tcosk[:], tsink[:]
                    )
                    nc.vector.tensor_copy(
                        kT2_sb[64:128, f0 : f0 + HS], kT2_sb[0:64, f0 : f0 + HS]
                    )
                    # V transpose to [token, d] for this half
                    for t in range(8 * hf, 8 * hf + 8):
                        trp = psT.tile([128, 64], BF, tag="tr", name=f"tr{t}")
                        nc.tensor.transpose(
                            trp[:],
                            vT_sb[:, 128 * t : 128 * (t + 1)],
                            ident_sb[0:64, 0:64],
                        )
                        nc.vector.tensor_copy(vaug_sb[:, t, 0:64], trp[:])
                nc.vector.memset(vaug_sb[:, :, 64:65], 1.0)

                for hf in range(2):
                    for p in range(2):
                        f0 = HS * hf
                        qp = psA.tile([128, HS], F32, tag="qkv", name=f"qp{p}_{hf}")
                        for sp in range(2):
                            for k in range(NCH):
                                nc.tensor.matmul(
                                    qp[:, SPAN * sp : SPAN * (sp + 1)],
                                    wq_sb[k][:, 128 * p : 128 * (p + 1)],
                                    xT_sb[k][:, f0 + SPAN * sp : f0 + SPAN * (sp + 1)],
                                    start=(k == 0),
                                    stop=(k == NCH - 1),
                                )
                        qb = ropep.tile([128, HS], BF, tag="qb", name=f"qb{p}{hf}")
                        nc.scalar.copy(qb[:], qp[:])
                        tcos = ropep.tile([128, HS], BF, tag="tcos", name=f"tc{p}{hf}")
                        tsin = ropep.tile([128, HS], BF, tag="tsin", name=f"ts{p}{hf}")
                        nc.vector.tensor_mul(tcos[:], qb[:], c2_sb[:, f0 : f0 + HS])
                        for dst, src in ((0, 32), (32, 0), (64, 96), (96, 64)):
                            nc.vector.tensor_mul(
                                tsin[dst : dst + 32, :],
                                qb[src : src + 32, :],
                                s2_sb[src : src + 32, f0 : f0 + HS],
                            )
                        nc.vector.tensor_add(
                            qT_sb[p][:, f0 : f0 + HS], tcos[:], tsin[:]
                        )

            # ---- phase 3: attention spans, AllGather, output projection ----
            with (
                tc.tile_pool(name="psS", bufs=2, space="PSUM") as psS,
                tc.tile_pool(name="psO", bufs=1, space="PSUM") as psO,
                tc.tile_pool(name="pp", bufs=5) as pp,
                tc.tile_pool(name="work", bufs=2) as work,
            ):
                rg = [[0, 1, 2, 3], [4, 5, 6, 7]]
                pending_oproj = []

                for J in range(NSPAN):
                    q0 = SPAN * J
                    nkt_j = 4 * (J + 1)
                    oproj_ready = pending_oproj
                    pending_oproj = []
                    opsum = psO.tile([128, 4 * SPAN], F32, tag="o", name=f"opsum{J}")

                    prev_pv = None  # (j, [pt_pr0, pt_pr1], col offset)

                    def emit_pv(j, pts, off):
                        for pr in range(2):
                            for hh in range(2):
                                h = 2 * pr + hh
                                nc.tensor.matmul(
                                    opsum[0:65, SPAN * h + off : SPAN * (h + 1)],
                                    vaug_sb[:, j, :],
                                    pts[pr][:, SPAN * hh + off : SPAN * (hh + 1)],
                                    start=(j == 0),
                                    stop=(j == nkt_j - 1),
                                )

                    for j in range(nkt_j):
                        jj = j - 4 * J  # >= 0 on causal-boundary k-tiles
                        off = 128 * jj if jj > 0 else 0
                        pts = []
                        for pr in range(2):
                            sps = psS.tile(
                                [128, 2 * SPAN], F32, tag="s", name=f"s{J}_{j}_{pr}"
                            )
                            pt = pp.tile(
                                [128, 2 * SPAN], BF, tag="p", name=f"p{J}_{j}_{pr}"
                            )
                            pts.append(pt)
                            src = qT_sb[pr]
                            nc.tensor.matmul(
                                sps[:, off:SPAN],
                                kT2_sb[0:64, 128 * j : 128 * (j + 1)],
                                src[0:64, q0 + off : q0 + SPAN],
                                start=True,
                                stop=True,
                            )
                            nc.tensor.matmul(
                                sps[:, SPAN + off : 2 * SPAN],
                                kT2_sb[64:128, 128 * j : 128 * (j + 1)],
                                src[64:128, q0 + off : q0 + SPAN],
                                start=True,
                                stop=True,
                            )
                            # exp over the two valid column blocks (strided AP)
                            nc.scalar.activation(
                                pt[:].rearrange("p (h q) -> p h q", h=2)[
                                    :, :, off:SPAN
                                ],
                                sps[:].rearrange("p (h q) -> p h q", h=2)[
                                    :, :, off:SPAN
                                ],
                                EXP,
                            )
                            if jj >= 0:
                                # causal triangle on the diagonal 128-block
                                af = nc.gpsimd.affine_select(
                                    pt[:].rearrange("p (h q) -> p h q", h=2)[
                                        :, :, off : off + 128
                                    ],
                                    pt[:].rearrange("p (h q) -> p h q", h=2)[
                                        :, :, off : off + 128
                                    ],
                                    pattern=[[0, 2], [1, 128]],
                                    compare_op=mybir.AluOpType.is_ge,
                                    fill=0.0,
                                    base=0,
                                    channel_multiplier=-1,
                                )
                                if first_gpsimd is not None:
                                    # pin the warmup collective to the front of
                                    # the gpsimd stream (order-only dep)
                                    tile.add_dep_helper(
                                        first_gpsimd[0].ins,
                                        af.ins,
                                        sync=False,
                                        reason="warmup AG first",
                                    )
                                    first_gpsimd = None
                        if prev_pv is not None:
                            emit_pv(*prev_pv)
                        prev_pv = (j, pts, off)
                        if j == 5:
                            for fn in oproj_ready:
                                fn()
                            oproj_ready = []
                    emit_pv(*prev_pv)

                    # normalization + AllGather + output projection, emitted
                    # deferred (inside the next span's j-loop) so the ACT
                    # recip chain and the AllGather hide behind attention
                    def make_norm_ag(J=J, q0=q0, opsum=opsum):
                        def _norm():
                            # denominator row -> SBUF bf16 (for the broadcast MM)
                            dsb = work.tile([1, 4 * SPAN], BF, tag="dsb", name=f"dsb{J}")
                            nc.scalar.copy(dsb[:], opsum[64:65, :])

                            def _bcag():
                                agin = dramp.tile([256, SPAN], BF, name=f"agin{J}")
                                agout = dramp.tile([4 * 256, SPAN], BF, name=f"agout{J}")
                                for h in range(4):
                                    bc = psS.tile(
                                        [64, SPAN], F32, tag="s", name=f"bc{J}_{h}"
                                    )
                                    nc.tensor.matmul(
                                        bc[:],
                                        ones_sb[:],
                                        dsb[0:1, SPAN * h : SPAN * (h + 1)],
                                        start=True,
                                        stop=True,
                                    )
                                    rec = work.tile(
                                        [64, SPAN], F32, tag="rec", name=f"rec{J}_{h}"
                                    )
                                    nc.vector.reciprocal_approx_fast(rec[:], bc[:])
                                    onrm = work.tile(
                                        [64, SPAN], BF, tag="onrm", name=f"on{J}_{h}"
                                    )
                                    nc.vector.tensor_mul(
                                        onrm[:],
                                        opsum[0:64, SPAN * h : SPAN * (h + 1)],
                                        rec[:],
                                    )
                                    nc.sync.dma_start(
                                        agin[64 * h : 64 * (h + 1), :], onrm[:]
                                    )

                                nc.gpsimd.collective_compute(
                                    "AllGather",
                                    mybir.AluOpType.bypass,
                                    replica_groups=rg,
                                    ins=[agin[:].opt()],
                                    outs=[agout[:].opt()],
                                )
                                ofull = work.tile(
                                    [128, NCH, SPAN], BF, tag="ofull", bufs=3, name=f"of{J}"
                                )
                                for k in range(NCH):
                                    nc.sync.dma_start(
                                        ofull[:, k, :], agout[128 * k : 128 * (k + 1), :]
                                    )

                                def _oproj():
                                    for half in range(2):
                                        po = psS.tile(
                                            [128, SPAN], F32, tag="s", name=f"po{J}_{half}"
                                        )
                                        for k in range(NCH):
                                            nc.tensor.matmul(
                                                po[:],
                                                wo_sb[k][
                                                    :, 128 * half : 128 * (half + 1)
                                                ],
                                                ofull[:, k, :],
                                                start=(k == 0),
                                                stop=(k == NCH - 1),
                                            )
                                        outT = work.tile(
                                            [128, SPAN],
                                            F32,
                                            tag="outT",
                                            name=f"ot{J}_{half}",
                                        )
                                        nc.vector.tensor_copy(outT[:], po[:])
                                        nc.sync.dma_start(
                                            out[
                                                128 * half : 128 * (half + 1),
                                                q0 : q0 + SPAN,
                                            ],
                                            outT[:],
                                        )

                                pending_oproj.append(_oproj)

                            _bcag()

                        return _norm

                    make_norm_ag()()

                for fn in oproj_ready:
                    fn()
                for fn in pending_oproj:
                    fn()

    nc.finalize()
    return nc


def _host_inputs(x, cos, sin, wq, wk, wv, wo):
    cosT = np.ascontiguousarray(cos.T).astype(np.float32)  # [64, S]
    sinT = np.ascontiguousarray(sin.T).astype(np.float32)
    s1n = np.concatenate([-sinT[0:32], sinT[32:64]], axis=0)  # [64, S]
    c2n = np.concatenate([cosT, cosT], axis=0).astype(BF16)  # [128, S]
    # partition-swapped: row p holds the sin factor for the partner row p^32,
    # so both DVE operands read from the same base partition
    s1w = np.concatenate([s1n[32:64], s1n[0:32]], axis=0)
    s2w = np.concatenate([s1w, s1w], axis=0).astype(BF16)
    cosT = cosT.astype(BF16)
    s1w = s1w.astype(BF16)
    ident = np.eye(128, dtype=BF16)

    in_maps = []
    for c in range(N_CORES):
        b, g = c // 4, c % 4
        xT = np.ascontiguousarray(x[b].T).astype(BF16)
        wq_c = np.ascontiguousarray(wq[:, 256 * g : 256 * (g + 1)] / 8.0).astype(BF16)
        wkv_c = np.ascontiguousarray(
            np.concatenate(
                [wk[:, 64 * g : 64 * (g + 1)], wv[:, 64 * g : 64 * (g + 1)]], axis=1
            )
        ).astype(BF16)
        wo_c = np.ascontiguousarray(wo[:, 256 * g : 256 * (g + 1)]).astype(BF16)
        in_maps.append(
            {
                "xT": xT,
                "wq": wq_c,
                "wkv": wkv_c,
                "wo": wo_c,
                "c2": c2n,
                "s2": s2w,
                "c1": cosT,
                "s1": s1w,
                "ident": ident,
            }
        )
    return in_maps


def kernel(x, cos, sin, wq, wk, wv, wo):
    if "nc" not in _CACHED:
        _CACHED["nc"] = _build_nc()
    nc = _CACHED["nc"]
    in_maps = _host_inputs(
        np.asarray(x, np.float32),
        np.asarray(cos, np.float32),
        np.asarray(sin, np.float32),
        np.asarray(wq, np.float32),
        np.asarray(wk, np.float32),
        np.asarray(wv, np.float32),
        np.asarray(wo, np.float32),
    )
    res = run_bass_kernel_spmd(
        nc, in_maps, core_ids=list(range(N_CORES)), trace=TRACE
    )
    _CACHED["last_result"] = res
    out = np.empty((B, S, HID), dtype=np.float32)
    for c in range(N_CORES):
        b, g = c // 4, c % 4
        out[b, :, 256 * g : 256 * (g + 1)] = res.results[c]["out"].T
    return out


# revision 14
# speedup vs baseline: 1.1933x; 1.0252x over previous
"""GQA attention block (RoPE + causal attention + output proj) on 8 TRN2 NeuronCores.

Sharding: batch (B=2) x kv-head-group (KVH=4) -> 8 cores.
Core c handles batch b=c//4, kv group g=c%4 (q heads 4g..4g+3, kv head g).
Per-core tensor-parallel attention; AllGather of per-head outputs within each
batch's 4-core group; column-split wo after the gather.

All matmuls run in bf16 (fp32 PSUM accumulation). Layouts are transposed
([feature, token]) so Q/K/V projections, scores (computed as S^T = K-stationary),
and PV all feed the TensorEngine without transposes; softmax runs without
max-subtraction (logits are provably small for this problem's scale).

Pipelining: PV runs one k-tile behind scores/exp; each span's output projection
is deferred until after the next span's attention so the AllGather latency hides
behind compute.
"""

import sys

if "/opt/trn_rl_repo" not in sys.path:
    sys.path.insert(0, "/opt/trn_rl_repo")

import numpy as np
import ml_dtypes

import concourse.bass as bass
import concourse.mybir as mybir
import concourse.tile as tile
from concourse import bacc
from concourse.bass_utils import run_bass_kernel_spmd

BF16 = ml_dtypes.bfloat16

B, S, HID = 2, 2048, 1024
H, KVH, D = 16, 4, 64
G = H // KVH
N_CORES = 8
SPAN = 512
NSPAN = S // SPAN  # 4
NCH = HID // 128  # 8 contraction chunks
NKT = S // 128  # 16 k-tiles
F32 = mybir.dt.float32
BF = mybir.dt.bfloat16

TRACE = False
_CACHED = {}


def _build_nc():
    nc = bacc.Bacc("TRN2", target_bir_lowering=False, debug=False, num_devices=N_CORES)

    xT = nc.dram_tensor("xT", [HID, S], BF, kind="ExternalInput")
    wq = nc.dram_tensor("wq", [HID, 256], BF, kind="ExternalInput")
    wkv = nc.dram_tensor("wkv", [HID, 128], BF, kind="ExternalInput")
    wo = nc.dram_tensor("wo", [HID, 256], BF, kind="ExternalInput")
    c2 = nc.dram_tensor("c2", [128, S], BF, kind="ExternalInput")
    s2 = nc.dram_tensor("s2", [128, S], BF, kind="ExternalInput")
    c1 = nc.dram_tensor("c1", [64, S], BF, kind="ExternalInput")
    s1 = nc.dram_tensor("s1", [64, S], BF, kind="ExternalInput")
    ident = nc.dram_tensor("ident", [128, 128], BF, kind="ExternalInput")
    out = nc.dram_tensor("out", [256, S], F32, kind="ExternalOutput")

    EXP = mybir.ActivationFunctionType.Exp
    LN = mybir.ActivationFunctionType.Ln

    with tile.TileContext(nc) as tc:
        with (
            tc.tile_pool(name="main", bufs=1) as main,
            tc.tile_pool(name="dramp", bufs=1, space="DRAM") as dramp,
        ):
            # ---- persistent SBUF; per-chunk input tiles so compute can start
            # as soon as each chunk's DMA lands ----
            xT_sb = [main.tile([128, S], BF, name=f"xT{k}") for k in range(NCH)]
            wq_sb = [main.tile([128, 256], BF, name=f"wq{k}") for k in range(NCH)]
            wkv_sb = [main.tile([128, 128], BF, name=f"wkv{k}") for k in range(NCH)]
            wo_sb = [main.tile([128, 256], BF, name=f"wo{k}") for k in range(NCH)]
            c2_sb = main.tile([128, S], BF, name="c2_sb")
            s2_sb = main.tile([128, S], BF, name="s2_sb")
            c1_sb = main.tile([64, S], BF, name="c1_sb")
            s1_sb = main.tile([64, S], BF, name="s1_sb")
            ident_sb = main.tile([128, 128], BF, name="ident_sb")
            qT0_sb = main.tile([128, S], BF, name="qT0_sb")
            qT1_sb = main.tile([128, S], BF, name="qT1_sb")
            kT2_sb = main.tile([128, S], BF, name="kT2_sb")
            vT_sb = main.tile([64, S], BF, name="vT_sb")
            vaug_sb = main.tile([128, NKT, 65], BF, name="vaug_sb")
            ones_sb = main.tile([1, 64], BF, name="ones_sb")
            qT_sb = [qT0_sb, qT1_sb]

            for k in range(NCH):
                nc.sync.dma_start(xT_sb[k][:], xT[128 * k : 128 * k + 128, :])
                nc.sync.dma_start(wkv_sb[k][:], wkv[128 * k : 128 * k + 128, :])
            for k in range(NCH):
                nc.sync.dma_start(wq_sb[k][:], wq[128 * k : 128 * k + 128, :])
            nc.sync.dma_start(c1_sb[:], c1[:])
            nc.sync.dma_start(s1_sb[:], s1[:])
            nc.sync.dma_start(c2_sb[:], c2[:])
            nc.sync.dma_start(s2_sb[:], s2[:])
            nc.sync.dma_start(ident_sb[:], ident[:])
            for k in range(NCH):
                nc.sync.dma_start(wo_sb[k][:], wo[128 * k : 128 * k + 128, :])
            nc.vector.memset(ones_sb[:], 1.0)

            # ---- phase 1: projections (transposed layout) + RoPE; KV first so
            # the V-transpose can run while the Q projections are still going ----
            HS = S // 2  # phase-1 half-sequence granularity (2 PSUM banks)
            with (
                tc.tile_pool(name="psA", bufs=2, space="PSUM") as psA,
                tc.tile_pool(name="ropep", bufs=2) as ropep,
                tc.tile_pool(name="psT", bufs=2, space="PSUM") as psT,
            ):
                for hf in range(2):
                    f0 = HS * hf
                    kvp = psA.tile([128, HS], F32, tag="qkv", name=f"kvp{hf}")
                    for sp in range(2):
                        for k in range(NCH):
                            nc.tensor.matmul(
                                kvp[:, SPAN * sp : SPAN * (sp + 1)],
                                wkv_sb[k][:],
                                xT_sb[k][:, f0 + SPAN * sp : f0 + SPAN * (sp + 1)],
                                start=(k == 0),
                                stop=(k == NCH - 1),
                            )
                    kb = ropep.tile([64, HS], BF, tag="kb", name=f"kb{hf}")
                    nc.scalar.copy(kb[:], kvp[0:64, :])
                    nc.scalar.copy(vT_sb[:, f0 : f0 + HS], kvp[64:128, :])
                    tcosk = ropep.tile([64, HS], BF, tag="tcos", name=f"tcosk{hf}")
                    tsink = ropep.tile([64, HS], BF, tag="tsin", name=f"tsink{hf}")
                    nc.vector.tensor_mul(tcosk[:], kb[:], c1_sb[:, f0 : f0 + HS])
                    for dst, src in ((0, 32), (32, 0)):
                        nc.vector.tensor_mul(
                            tsink[dst : dst + 32, :],
                            kb[src : src + 32, :],
                            s1_sb[src : src + 32, f0 : f0 + HS],
                        )
                    nc.vector.tensor_add(
                        kT2_sb[0:64, f0 : f0 + HS], tcosk[:], tsink[:]
                    )
                    nc.vector.tensor_copy(
                        kT2_sb[64:128, f0 : f0 + HS], kT2_sb[0:64, f0 : f0 + HS]
                    )
                    # V transpose to [token, d] for this half
                    for t in range(8 * hf, 8 * hf + 8):
                        trp = psT.tile([128, 64], BF, tag="tr", name=f"tr{t}")
                        nc.tensor.transpose(
                            trp[:],
                            vT_sb[:, 128 * t : 128 * (t + 1)],
                            ident_sb[0:64, 0:64],
                        )
                        nc.vector.tensor_copy(vaug_sb[:, t, 0:64], trp[:])
                nc.vector.memset(vaug_sb[:, :, 64:65], 1.0)

                for hf in range(2):
                    for p in range(2):
                        f0 = HS * hf
                        qp = psA.tile([128, HS], F32, tag="qkv", name=f"qp{p}_{hf}")
                        for sp in range(2):
                            for k in range(NCH):
                                nc.tensor.matmul(
                                    qp[:, SPAN * sp : SPAN * (sp + 1)],
                                    wq_sb[k][:, 128 * p : 128 * (p + 1)],
                                    xT_sb[k][:, f0 + SPAN * sp : f0 + SPAN * (sp + 1)],
                                    start=(k == 0),
                                    stop=(k == NCH - 1),
                                )
                        qb = ropep.tile([128, HS], BF, tag="qb", name=f"qb{p}{hf}")
                        nc.scalar.copy(qb[:], qp[:])
                        tcos = ropep.tile([128, HS], BF, tag="tcos", name=f"tc{p}{hf}")
                        tsin = ropep.tile([128, HS], BF, tag="tsin", name=f"ts{p}{hf}")
                        nc.vector.tensor_mul(tcos[:], qb[:], c2_sb[:, f0 : f0 + HS])
                        for dst, src in ((0, 32), (32, 0), (64, 96), (96, 64)):
                            nc.vector.tensor_mul(
                                tsin[dst : dst + 32, :],
                                qb[src : src + 32, :],
                                s2_sb[src : src + 32, f0 : f0 + HS],
                            )
                        nc.vector.tensor_add(
                            qT_sb[p][:, f0 : f0 + HS], tcos[:], tsin[:]
                        )

            # ---- phase 3: attention spans, AllGather, output projection ----
            with (
                tc.tile_pool(name="psS", bufs=2, space="PSUM") as psS,
                tc.tile_pool(name="psO", bufs=1, space="PSUM") as psO,
                tc.tile_pool(name="pp", bufs=5) as pp,
                tc.tile_pool(name="work", bufs=2) as work,
            ):
                rg = [[0, 1, 2, 3], [4, 5, 6, 7]]
                pending_oproj = []

                for J in range(NSPAN):
                    q0 = SPAN * J
                    nkt_j = 4 * (J + 1)
                    oproj_ready = pending_oproj
                    pending_oproj = []
                    opsum = psO.tile([128, 4 * SPAN], F32, tag="o", name=f"opsum{J}")

                    prev_pv = None  # (j, [pt_pr0, pt_pr1], col offset)

                    def emit_pv(j, pts, off):
                        for pr in range(2):
                            for hh in range(2):
                                h = 2 * pr + hh
                                nc.tensor.matmul(
                                    opsum[0:65, SPAN * h + off : SPAN * (h + 1)],
                                    vaug_sb[:, j, :],
                                    pts[pr][:, SPAN * hh + off : SPAN * (hh + 1)],
                                    start=(j == 0),
                                    stop=(j == nkt_j - 1),
                                )

                    for j in range(nkt_j):
                        jj = j - 4 * J  # >= 0 on causal-boundary k-tiles
                        off = 128 * jj if jj > 0 else 0
                        pts = []
                        for pr in range(2):
                            sps = psS.tile(
                                [128, 2 * SPAN], F32, tag="s", name=f"s{J}_{j}_{pr}"
                            )
                            pt = pp.tile(
                                [128, 2 * SPAN], BF, tag="p", name=f"p{J}_{j}_{pr}"
                            )
                            pts.append(pt)
                            src = qT_sb[pr]
                            nc.tensor.matmul(
                                sps[:, off:SPAN],
                                kT2_sb[0:64, 128 * j : 128 * (j + 1)],
                                src[0:64, q0 + off : q0 + SPAN],
                                start=True,
                                stop=True,
                            )
                            nc.tensor.matmul(
                                sps[:, SPAN + off : 2 * SPAN],
                                kT2_sb[64:128, 128 * j : 128 * (j + 1)],
                                src[64:128, q0 + off : q0 + SPAN],
                                start=True,
                                stop=True,
                            )
                            # exp over the two valid column blocks (strided AP)
                            nc.scalar.activation(
                                pt[:].rearrange("p (h q) -> p h q", h=2)[
                                    :, :, off:SPAN
                                ],
                                sps[:].rearrange("p (h q) -> p h q", h=2)[
                                    :, :, off:SPAN
                                ],
                                EXP,
                            )
                            if jj >= 0:
                                # causal triangle on the diagonal 128-block
                                af = nc.gpsimd.affine_select(
                                    pt[:].rearrange("p (h q) -> p h q", h=2)[
                                        :, :, off : off + 128
                                    ],
                                    pt[:].rearrange("p (h q) -> p h q", h=2)[
                                        :, :, off : off + 128
                                    ],
                                    pattern=[[0, 2], [1, 128]],
                                    compare_op=mybir.AluOpType.is_ge,
                                    fill=0.0,
                                    base=0,
                                    channel_multiplier=-1,
                                )
                        if prev_pv is not None:
                            emit_pv(*prev_pv)
                        prev_pv = (j, pts, off)
                        if j == 5:
                            for fn in oproj_ready:
                                fn()
                            oproj_ready = []
                    emit_pv(*prev_pv)

                    # normalization + AllGather + output projection, emitted
                    # deferred (inside the next span's j-loop) so the ACT
                    # recip chain and the AllGather hide behind attention
                    def make_norm_ag(J=J, q0=q0, opsum=opsum):
                        def _norm():
                            # denominator row -> SBUF bf16 (for the broadcast MM)
                            dsb = work.tile([1, 4 * SPAN], BF, tag="dsb", name=f"dsb{J}")
                            nc.scalar.copy(dsb[:], opsum[64:65, :])

                            def _bcag():
                                agin = dramp.tile([256, SPAN], BF, name=f"agin{J}")
                                agout = dramp.tile([4 * 256, SPAN], BF, name=f"agout{J}")
                                for h in range(4):
                                    bc = psS.tile(
                                        [64, SPAN], F32, tag="s", name=f"bc{J}_{h}"
                                    )
                                    nc.tensor.matmul(
                                        bc[:],
                                        ones_sb[:],
                                        dsb[0:1, SPAN * h : SPAN * (h + 1)],
                                        start=True,
                                        stop=True,
                                    )
                                    rec = work.tile(
                                        [64, SPAN], F32, tag="rec", name=f"rec{J}_{h}"
                                    )
                                    nc.vector.reciprocal_approx_fast(rec[:], bc[:])
                                    onrm = work.tile(
                                        [64, SPAN], BF, tag="onrm", name=f"on{J}_{h}"
                                    )
                                    nc.vector.tensor_mul(
                                        onrm[:],
                                        opsum[0:64, SPAN * h : SPAN * (h + 1)],
                                        rec[:],
                                    )
                                    nc.sync.dma_start(
                                        agin[64 * h : 64 * (h + 1), :], onrm[:]
                                    )

                                nc.gpsimd.collective_compute(
                                    "AllGather",
                                    mybir.AluOpType.bypass,
                                    replica_groups=rg,
                                    ins=[agin[:].opt()],
                                    outs=[agout[:].opt()],
                                )
                                ofull = work.tile(
                                    [128, NCH, SPAN], BF, tag="ofull", bufs=3, name=f"of{J}"
                                )
                                for k in range(NCH):
                                    nc.sync.dma_start(
                                        ofull[:, k, :], agout[128 * k : 128 * (k + 1), :]
                                    )

                                def _oproj():
                                    for half in range(2):
                                        po = psS.tile(
                                            [128, SPAN], F32, tag="s", name=f"po{J}_{half}"
                                        )
                                        for k in range(NCH):
                                            nc.tensor.matmul(
                                                po[:],
                                                wo_sb[k][
                                                    :, 128 * half : 128 * (half + 1)
                                                ],
                                                ofull[:, k, :],
                                                start=(k == 0),
                                                stop=(k == NCH - 1),
                                            )
                                        outT = work.tile(
                                            [128, SPAN],
                                            F32,
                                            tag="outT",
                                            name=f"ot{J}_{half}",
                                        )
                                        nc.vector.tensor_copy(outT[:], po[:])
                                        nc.sync.dma_start(
                                            out[
                                                128 * half : 128 * (half + 1),
                                                q0 : q0 + SPAN,
                                            ],
                                            outT[:],
                                        )

                                pending_oproj.append(_oproj)

                            _bcag()

                        return _norm

                    make_norm_ag()()

                for fn in oproj_ready:
                    fn()
                for fn in pending_oproj:
                    fn()

    nc.finalize()
    return nc


def _host_inputs(x, cos, sin, wq, wk, wv, wo):
    cosT = np.ascontiguousarray(cos.T).astype(np.float32)  # [64, S]
    sinT = np.ascontiguousarray(sin.T).astype(np.float32)
    s1n = np.concatenate([-sinT[0:32], sinT[32:64]], axis=0)  # [64, S]
    c2n = np.concatenate([cosT, cosT], axis=0).astype(BF16)  # [128, S]
    # partition-swapped: row p holds the sin factor for the partner row p^32,
    # so both DVE operands read from the same base partition
    s1w = np.concatenate([s1n[32:64], s1n[0:32]], axis=0)
    s2w = np.concatenate([s1w, s1w], axis=0).astype(BF16)
    cosT = cosT.astype(BF16)
    s1w = s1w.astype(BF16)
    ident = np.eye(128, dtype=BF16)

    in_maps = []
    for c in range(N_CORES):
        b, g = c // 4, c % 4
        xT = np.ascontiguousarray(x[b].T).astype(BF16)
        wq_c = np.ascontiguousarray(wq[:, 256 * g : 256 * (g + 1)] / 8.0).astype(BF16)
        wkv_c = np.ascontiguousarray(
            np.concatenate(
                [wk[:, 64 * g : 64 * (g + 1)], wv[:, 64 * g : 64 * (g + 1)]], axis=1
            )
        ).astype(BF16)
        wo_c = np.ascontiguousarray(wo[:, 256 * g : 256 * (g + 1)]).astype(BF16)
        in_maps.append(
            {
                "xT": xT,
                "wq": wq_c,
                "wkv": wkv_c,
                "wo": wo_c,
                "c2": c2n,
                "s2": s2w,
                "c1": cosT,
                "s1": s1w,
                "ident": ident,
            }
        )
    return in_maps


def kernel(x, cos, sin, wq, wk, wv, wo):
    if "nc" not in _CACHED:
        _CACHED["nc"] = _build_nc()
    nc = _CACHED["nc"]
    in_maps = _host_inputs(
        np.asarray(x, np.float32),
        np.asarray(cos, np.float32),
        np.asarray(sin, np.float32),
        np.asarray(wq, np.float32),
        np.asarray(wk, np.float32),
        np.asarray(wv, np.float32),
        np.asarray(wo, np.float32),
    )
    res = run_bass_kernel_spmd(
        nc, in_maps, core_ids=list(range(N_CORES)), trace=TRACE
    )
    _CACHED["last_result"] = res
    out = np.empty((B, S, HID), dtype=np.float32)
    for c in range(N_CORES):
        b, g = c // 4, c % 4
        out[b, :, 256 * g : 256 * (g + 1)] = res.results[c]["out"].T
    return out


# revision 15
# speedup vs baseline: 1.3183x; 1.1048x over previous
"""GQA attention block (RoPE + causal attention + output proj) on 8 TRN2 NeuronCores.

Sharding: batch (B=2) x kv-head-group (KVH=4) -> 8 cores.
Core c handles batch b=c//4, kv group g=c%4 (q heads 4g..4g+3, kv head g).
Per-core tensor-parallel attention; AllGather of per-head outputs within each
batch's 4-core group; column-split wo after the gather.

All matmuls run in bf16 (fp32 PSUM accumulation). Layouts are transposed
([feature, token]) so Q/K/V projections, scores (computed as S^T = K-stationary),
and PV all feed the TensorEngine without transposes; softmax runs without
max-subtraction (logits are provably small for this problem's scale).

Pipelining: PV runs one k-tile behind scores/exp; each span's output projection
is deferred until after the next span's attention so the AllGather latency hides
behind compute.
"""

import sys

if "/opt/trn_rl_repo" not in sys.path:
    sys.path.insert(0, "/opt/trn_rl_repo")

import numpy as np
import ml_dtypes

import concourse.bass as bass
import concourse.mybir as mybir
import concourse.tile as tile
from concourse import bacc
from concourse.bass_utils import run_bass_kernel_spmd

BF16 = ml_dtypes.bfloat16

B, S, HID = 2, 2048, 1024
H, KVH, D = 16, 4, 64
G = H // KVH
N_CORES = 8
SPAN = 512
NSPAN = S // SPAN  # 4
NCH = HID // 128  # 8 contraction chunks
NKT = S // 128  # 16 k-tiles
F32 = mybir.dt.float32
BF = mybir.dt.bfloat16

TRACE = False
_CACHED = {}


def _build_nc():
    nc = bacc.Bacc("TRN2", target_bir_lowering=False, debug=False, num_devices=N_CORES)

    xT = nc.dram_tensor("xT", [HID, S], BF, kind="ExternalInput")
    wq = nc.dram_tensor("wq", [HID, 256], BF, kind="ExternalInput")
    wkv = nc.dram_tensor("wkv", [HID, 128], BF, kind="ExternalInput")
    wo = nc.dram_tensor("wo", [HID, 256], BF, kind="ExternalInput")
    c2 = nc.dram_tensor("c2", [128, S], BF, kind="ExternalInput")
    s2 = nc.dram_tensor("s2", [128, S], BF, kind="ExternalInput")
    c1 = nc.dram_tensor("c1", [64, S], BF, kind="ExternalInput")
    s1 = nc.dram_tensor("s1", [64, S], BF, kind="ExternalInput")
    ident = nc.dram_tensor("ident", [128, 128], BF, kind="ExternalInput")
    out = nc.dram_tensor("out", [256, S], F32, kind="ExternalOutput")

    EXP = mybir.ActivationFunctionType.Exp
    LN = mybir.ActivationFunctionType.Ln

    with tile.TileContext(nc) as tc:
        with (
            tc.tile_pool(name="main", bufs=1) as main,
            tc.tile_pool(name="dramp", bufs=1, space="DRAM") as dramp,
        ):
            # ---- persistent SBUF; per-chunk input tiles so compute can start
            # as soon as each chunk's DMA lands ----
            xT_sb = [main.tile([128, S], BF, name=f"xT{k}") for k in range(NCH)]
            wq_sb = [main.tile([128, 256], BF, name=f"wq{k}") for k in range(NCH)]
            wkv_sb = [main.tile([128, 128], BF, name=f"wkv{k}") for k in range(NCH)]
            wo_sb = [main.tile([128, 256], BF, name=f"wo{k}") for k in range(NCH)]
            c2_sb = main.tile([128, S], BF, name="c2_sb")
            s2_sb = main.tile([128, S], BF, name="s2_sb")
            c1_sb = main.tile([64, S], BF, name="c1_sb")
            s1_sb = main.tile([64, S], BF, name="s1_sb")
            ident_sb = main.tile([128, 128], BF, name="ident_sb")
            qT0_sb = main.tile([128, S], BF, name="qT0_sb")
            qT1_sb = main.tile([128, S], BF, name="qT1_sb")
            kT2_sb = main.tile([128, S], BF, name="kT2_sb")
            vT_sb = main.tile([64, S], BF, name="vT_sb")
            vaug_sb = main.tile([128, NKT, 65], BF, name="vaug_sb")
            ones_sb = main.tile([1, 64], BF, name="ones_sb")
            qT_sb = [qT0_sb, qT1_sb]

            for k in range(NCH):
                nc.sync.dma_start(xT_sb[k][:], xT[128 * k : 128 * k + 128, :])
                nc.sync.dma_start(wkv_sb[k][:], wkv[128 * k : 128 * k + 128, :])
            for k in range(NCH):
                nc.sync.dma_start(wq_sb[k][:], wq[128 * k : 128 * k + 128, :])
            nc.sync.dma_start(c1_sb[:], c1[:])
            nc.sync.dma_start(s1_sb[:], s1[:])
            nc.sync.dma_start(c2_sb[:], c2[:])
            nc.sync.dma_start(s2_sb[:], s2[:])
            nc.sync.dma_start(ident_sb[:], ident[:])
            for k in range(NCH):
                nc.sync.dma_start(wo_sb[k][:], wo[128 * k : 128 * k + 128, :])
            nc.vector.memset(ones_sb[:], 1.0)

            # ---- phase 1: projections (transposed layout) + RoPE; KV first so
            # the V-transpose can run while the Q projections are still going ----
            HS = S // 2  # phase-1 half-sequence granularity (2 PSUM banks)
            with (
                tc.tile_pool(name="psA", bufs=2, space="PSUM") as psA,
                tc.tile_pool(name="ropep", bufs=2) as ropep,
                tc.tile_pool(name="psT", bufs=2, space="PSUM") as psT,
            ):
                for hf in range(2):
                    f0 = HS * hf
                    kvp = psA.tile([128, HS], F32, tag="qkv", name=f"kvp{hf}")
                    for sp in range(2):
                        for k in range(NCH):
                            nc.tensor.matmul(
                                kvp[:, SPAN * sp : SPAN * (sp + 1)],
                                wkv_sb[k][:],
                                xT_sb[k][:, f0 + SPAN * sp : f0 + SPAN * (sp + 1)],
                                start=(k == 0),
                                stop=(k == NCH - 1),
                            )
                    kb = ropep.tile([64, HS], BF, tag="kb", name=f"kb{hf}")
                    nc.scalar.copy(kb[:], kvp[0:64, :])
                    nc.scalar.copy(vT_sb[:, f0 : f0 + HS], kvp[64:128, :])
                    tcosk = ropep.tile([64, HS], BF, tag="tcos", name=f"tcosk{hf}")
                    tsink = ropep.tile([64, HS], BF, tag="tsin", name=f"tsink{hf}")
                    nc.vector.tensor_mul(tcosk[:], kb[:], c1_sb[:, f0 : f0 + HS])
                    for dst, src in ((0, 32), (32, 0)):
                        nc.vector.tensor_mul(
                            tsink[dst : dst + 32, :],
                            kb[src : src + 32, :],
                            s1_sb[src : src + 32, f0 : f0 + HS],
                        )
                    nc.vector.tensor_add(
                        kT2_sb[0:64, f0 : f0 + HS], tcosk[:], tsink[:]
                    )
                    nc.vector.tensor_copy(
                        kT2_sb[64:128, f0 : f0 + HS], kT2_sb[0:64, f0 : f0 + HS]
                    )
                    # V transpose to [token, d] for this half
                    for t in range(8 * hf, 8 * hf + 8):
                        trp = psT.tile([128, 64], BF, tag="tr", name=f"tr{t}")
                        nc.tensor.transpose(
                            trp[:],
                            vT_sb[:, 128 * t : 128 * (t + 1)],
                            ident_sb[0:64, 0:64],
                        )
                        nc.vector.tensor_copy(vaug_sb[:, t, 0:64], trp[:])
                nc.vector.memset(vaug_sb[:, :, 64:65], 1.0)

                for hf in range(2):
                    for p in range(2):
                        f0 = HS * hf
                        qp = psA.tile([128, HS], F32, tag="qkv", name=f"qp{p}_{hf}")
                        for sp in range(2):
                            for k in range(NCH):
                                nc.tensor.matmul(
                                    qp[:, SPAN * sp : SPAN * (sp + 1)],
                                    wq_sb[k][:, 128 * p : 128 * (p + 1)],
                                    xT_sb[k][:, f0 + SPAN * sp : f0 + SPAN * (sp + 1)],
                                    start=(k == 0),
                                    stop=(k == NCH - 1),
                                )
                        qb = ropep.tile([128, HS], BF, tag="qb", name=f"qb{p}{hf}")
                        nc.scalar.copy(qb[:], qp[:])
                        tcos = ropep.tile([128, HS], BF, tag="tcos", name=f"tc{p}{hf}")
                        tsin = ropep.tile([128, HS], BF, tag="tsin", name=f"ts{p}{hf}")
                        nc.vector.tensor_mul(tcos[:], qb[:], c2_sb[:, f0 : f0 + HS])
                        for dst, src in ((0, 32), (32, 0), (64, 96), (96, 64)):
                            nc.vector.tensor_mul(
                                tsin[dst : dst + 32, :],
                                qb[src : src + 32, :],
                                s2_sb[src : src + 32, f0 : f0 + HS],
                            )
                        nc.vector.tensor_add(
                            qT_sb[p][:, f0 : f0 + HS], tcos[:], tsin[:]
                        )

            # ---- phase 3: attention spans, AllGather, output projection ----
            with (
                tc.tile_pool(name="psS", bufs=2, space="PSUM") as psS,
                tc.tile_pool(name="psO", bufs=1, space="PSUM") as psO,
                tc.tile_pool(name="pp", bufs=5) as pp,
                tc.tile_pool(name="work", bufs=2) as work,
            ):
                rg = [[0, 1, 2, 3], [4, 5, 6, 7]]
                pending_oproj = []
                pending_norm = []

                for J in range(NSPAN):
                    q0 = SPAN * J
                    nkt_j = 4 * (J + 1)
                    oproj_ready = pending_oproj
                    pending_oproj = []
                    opsum = psO.tile([128, 4 * SPAN], F32, tag="o", name=f"opsum{J}")

                    prev_pv = None  # (j, [pt_pr0, pt_pr1], col offset)

                    def emit_pv(j, pts, off):
                        for pr in range(2):
                            for hh in range(2):
                                h = 2 * pr + hh
                                nc.tensor.matmul(
                                    opsum[0:65, SPAN * h + off : SPAN * (h + 1)],
                                    vaug_sb[:, j, :],
                                    pts[pr][:, SPAN * hh + off : SPAN * (hh + 1)],
                                    start=(j == 0),
                                    stop=(j == nkt_j - 1),
                                )

                    for j in range(nkt_j):
                        jj = j - 4 * J  # >= 0 on causal-boundary k-tiles
                        off = 128 * jj if jj > 0 else 0
                        pts = []
                        for pr in range(2):
                            sps = psS.tile(
                                [128, 2 * SPAN], F32, tag="s", name=f"s{J}_{j}_{pr}"
                            )
                            pt = pp.tile(
                                [128, 2 * SPAN], BF, tag="p", name=f"p{J}_{j}_{pr}"
                            )
                            pts.append(pt)
                            src = qT_sb[pr]
                            nc.tensor.matmul(
                                sps[:, off:SPAN],
                                kT2_sb[0:64, 128 * j : 128 * (j + 1)],
                                src[0:64, q0 + off : q0 + SPAN],
                                start=True,
                                stop=True,
                            )
                            nc.tensor.matmul(
                                sps[:, SPAN + off : 2 * SPAN],
                                kT2_sb[64:128, 128 * j : 128 * (j + 1)],
                                src[64:128, q0 + off : q0 + SPAN],
                                start=True,
                                stop=True,
                            )
                            # exp over the two valid column blocks (strided AP)
                            nc.scalar.activation(
                                pt[:].rearrange("p (h q) -> p h q", h=2)[
                                    :, :, off:SPAN
                                ],
                                sps[:].rearrange("p (h q) -> p h q", h=2)[
                                    :, :, off:SPAN
                                ],
                                EXP,
                            )
                            if jj >= 0:
                                # causal triangle on the diagonal 128-block
                                af = nc.gpsimd.affine_select(
                                    pt[:].rearrange("p (h q) -> p h q", h=2)[
                                        :, :, off : off + 128
                                    ],
                                    pt[:].rearrange("p (h q) -> p h q", h=2)[
                                        :, :, off : off + 128
                                    ],
                                    pattern=[[0, 2], [1, 128]],
                                    compare_op=mybir.AluOpType.is_ge,
                                    fill=0.0,
                                    base=0,
                                    channel_multiplier=-1,
                                )
                        if prev_pv is not None:
                            emit_pv(*prev_pv)
                        prev_pv = (j, pts, off)
                        if j == 1:
                            for fn in pending_norm:
                                fn()
                            pending_norm = []
                        if j == 5:
                            for fn in oproj_ready:
                                fn()
                            oproj_ready = []
                    emit_pv(*prev_pv)

                    # normalization + AllGather + output projection, emitted
                    # deferred (inside the next span's j-loop) so the ACT
                    # recip chain and the AllGather hide behind attention
                    def make_norm_ag(J=J, q0=q0, opsum=opsum):
                        def _norm():
                            # denominator row -> SBUF bf16 (for the broadcast MM)
                            dsb = work.tile([1, 4 * SPAN], BF, tag="dsb", name=f"dsb{J}")
                            nc.scalar.copy(dsb[:], opsum[64:65, :])

                            def _bcag():
                                agin = dramp.tile([256, SPAN], BF, name=f"agin{J}")
                                agout = dramp.tile([4 * 256, SPAN], BF, name=f"agout{J}")
                                for h in range(4):
                                    bc = psS.tile(
                                        [64, SPAN], F32, tag="s", name=f"bc{J}_{h}"
                                    )
                                    nc.tensor.matmul(
                                        bc[:],
                                        ones_sb[:],
                                        dsb[0:1, SPAN * h : SPAN * (h + 1)],
                                        start=True,
                                        stop=True,
                                    )
                                    rec = work.tile(
                                        [64, SPAN], F32, tag="rec", name=f"rec{J}_{h}"
                                    )
                                    nc.vector.reciprocal_approx_fast(rec[:], bc[:])
                                    onrm = work.tile(
                                        [64, SPAN], BF, tag="onrm", name=f"on{J}_{h}"
                                    )
                                    nc.vector.tensor_mul(
                                        onrm[:],
                                        opsum[0:64, SPAN * h : SPAN * (h + 1)],
                                        rec[:],
                                    )
                                    nc.sync.dma_start(
                                        agin[64 * h : 64 * (h + 1), :], onrm[:]
                                    )

                                nc.gpsimd.collective_compute(
                                    "AllGather",
                                    mybir.AluOpType.bypass,
                                    replica_groups=rg,
                                    ins=[agin[:].opt()],
                                    outs=[agout[:].opt()],
                                )
                                ofull = work.tile(
                                    [128, NCH, SPAN], BF, tag="ofull", bufs=3, name=f"of{J}"
                                )
                                for k in range(NCH):
                                    nc.sync.dma_start(
                                        ofull[:, k, :], agout[128 * k : 128 * (k + 1), :]
                                    )

                                def _oproj():
                                    for half in range(2):
                                        po = psS.tile(
                                            [128, SPAN], F32, tag="s", name=f"po{J}_{half}"
                                        )
                                        for k in range(NCH):
                                            nc.tensor.matmul(
                                                po[:],
                                                wo_sb[k][
                                                    :, 128 * half : 128 * (half + 1)
                                                ],
                                                ofull[:, k, :],
                                                start=(k == 0),
                                                stop=(k == NCH - 1),
                                            )
                                        outT = work.tile(
                                            [128, SPAN],
                                            F32,
                                            tag="outT",
                                            name=f"ot{J}_{half}",
                                        )
                                        nc.vector.tensor_copy(outT[:], po[:])
                                        nc.sync.dma_start(
                                            out[
                                                128 * half : 128 * (half + 1),
                                                q0 : q0 + SPAN,
                                            ],
                                            outT[:],
                                        )

                                pending_oproj.append(_oproj)

                            _bcag()

                        return _norm

                    pending_norm.append(make_norm_ag())

                for fn in pending_norm:
                    fn()
                for fn in oproj_ready:
                    fn()
                for fn in pending_oproj:
                    fn()

    nc.finalize()
    return nc


def _host_inputs(x, cos, sin, wq, wk, wv, wo):
    cosT = np.ascontiguousarray(cos.T).astype(np.float32)  # [64, S]
    sinT = np.ascontiguousarray(sin.T).astype(np.float32)
    s1n = np.concatenate([-sinT[0:32], sinT[32:64]], axis=0)  # [64, S]
    c2n = np.concatenate([cosT, cosT], axis=0).astype(BF16)  # [128, S]
    # partition-swapped: row p holds the sin factor for the partner row p^32,
    # so both DVE operands read from the same base partition
    s1w = np.concatenate([s1n[32:64], s1n[0:32]], axis=0)
    s2w = np.concatenate([s1w, s1w], axis=0).astype(BF16)
    cosT = cosT.astype(BF16)
    s1w = s1w.astype(BF16)
    ident = np.eye(128, dtype=BF16)

    in_maps = []
    for c in range(N_CORES):
        b, g = c // 4, c % 4
        xT = np.ascontiguousarray(x[b].T).astype(BF16)
        wq_c = np.ascontiguousarray(wq[:, 256 * g : 256 * (g + 1)] / 8.0).astype(BF16)
        wkv_c = np.ascontiguousarray(
            np.concatenate(
                [wk[:, 64 * g : 64 * (g + 1)], wv[:, 64 * g : 64 * (g + 1)]], axis=1
            )
        ).astype(BF16)
        wo_c = np.ascontiguousarray(wo[:, 256 * g : 256 * (g + 1)]).astype(BF16)
        in_maps.append(
            {
                "xT": xT,
                "wq": wq_c,
                "wkv": wkv_c,
                "wo": wo_c,
                "c2": c2n,
                "s2": s2w,
                "c1": cosT,
                "s1": s1w,
                "ident": ident,
            }
        )
    return in_maps


def kernel(x, cos, sin, wq, wk, wv, wo):
    if "nc" not in _CACHED:
        _CACHED["nc"] = _build_nc()
    nc = _CACHED["nc"]
    in_maps = _host_inputs(
        np.asarray(x, np.float32),
        np.asarray(cos, np.float32),
        np.asarray(sin, np.float32),
        np.asarray(wq, np.float32),
        np.asarray(wk, np.float32),
        np.asarray(wv, np.float32),
        np.asarray(wo, np.float32),
    )
    res = run_bass_kernel_spmd(
        nc, in_maps, core_ids=list(range(N_CORES)), trace=TRACE
    )
    _CACHED["last_result"] = res
    out = np.empty((B, S, HID), dtype=np.float32)
    for c in range(N_CORES):
        b, g = c // 4, c % 4
        out[b, :, 256 * g : 256 * (g + 1)] = res.results[c]["out"].T
    return out
